# revision 25
# baseline (speedup 1.0000x reference)
"""CARAFE-downsample Trainium2 kernel (B=8, C=256, H=W=128, K=5, S=2, M=64).

Sharding: data-parallel over batch B across 8 NeuronCores (one sample per core).

V5: the cost model (and HW) serialize all DMA transfers on a shared DMA-engine
pool, so the design minimizes TOTAL DMA bytes:
  - x loaded once per core as fp16 [256, 16384] (8.4 MB, the floor)
  - pixel-major xt built with PE (TensorE) transposes + PSUM->SBUF copies
    (no DMA), staggered 16 rows per sub
  - A-matrix: per-partition scatter writes only the 5 band rows each output
    row actually uses (5*128 slots/pair, 29% less than the 7-row band), one
    xbar DMA transpose per sub
  - encoder conv computed transposed (out[px, 25] per pair, contract M):
    25 columns streamed per matmul instead of 512; softmax needs no PE
    transposes; b2 added via a small const tile on DVE
  - weighted sum: 10 64-col accumulating matmuls per (pair, cc) against the
    compact 5-row amat (two disjoint PSUM accumulation chains per half)

Engine budget per core (cost model): DMA ~32+18, PE ~50, ACT/DVE ~20-30,
Pool ~17.

Pipeline: produce conv/softmax/scatter/A-transpose for sub sb, PE-transpose
the next 16 x-rows, then weighted-sum consume of sub sb-2. Output stored fp16
scrambled, unscrambled on host.
"""

import sys

if "/opt/trn_rl_repo" not in sys.path:
    sys.path.insert(0, "/opt/trn_rl_repo")

import numpy as np

import concourse.bacc as bacc
import concourse.tile as tile
from concourse import mybir
from concourse.bass_utils import run_bass_kernel_spmd

F32 = mybir.dt.float32
F16 = mybir.dt.float16
I16 = mybir.dt.int16

B, C, H, W = 8, 256, 128, 128
M = 64          # compressed channels
K = 5           # carafe kernel size
S = 2           # stride
KK = K * K      # 25
nH, nW = H // S, W // S          # 64, 64
NPIX = H * W                     # 16384
QCH = 7                          # band rows per pair (4P-2 .. 4P+4)
GSLOT = QCH * W                  # 896 at-slots per pair
NIDX = 104                       # scatter idx cols (4 pairs x 26)


def _exp_taps5():
    """Constant scatter index map for one sub (4 pairs): [128, 104] int16.

    Partition p = s*64+ow (s = output row within pair). Col 26*h+k -> pair h's
    half-scatter slot (h%2)*896 + (2s+i)*128 + (2*ow+j-2) for k=(i,j); col
    26*h+25 unused (-1). Scatters run per pair-group (2 pairs, 1792 elems).
    """
    idx = np.full((128, NIDX), -1, dtype=np.int16)
    for p in range(128):
        s, ow = p // nW, p % nW
        for k in range(KK):
            i, j = k // K, k % K
            iw = 2 * ow + j - 2
            if 0 <= iw < W:
                slot = (2 * s + i) * W + iw
                for h in range(4):
                    idx[p, 26 * h + k] = (h % 2) * GSLOT + slot
    return idx


def _build_program(repeat=1):
    nc = bacc.Bacc("TRN2", target_bir_lowering=False, debug=False, num_devices=8)

    xcm_d = nc.dram_tensor("xcm", [C, NPIX], F16, kind="ExternalInput")
    # packed consts: cpk f16 [128, 496] = w1t(0:128) | id128(128:256) |
    # w2t(256:481, partitions 0:64); bpk f32 [128, 101] = b2r(0:100) |
    # b1(100, partitions 0:64)
    cpk_d = nc.dram_tensor("cpk", [128, 496], F16, kind="ExternalInput")
    bpk_d = nc.dram_tensor("bpk", [128, 101], F32, kind="ExternalInput")
    ix_d = nc.dram_tensor("scidx", [128, NIDX], I16, kind="ExternalInput")
    # out[p, q, gl, pr, cc, s, ow] fp16; host maps c = cc*128+p,
    # oh = 16q + 4gl + 2pr + s.
    o_d = nc.dram_tensor("out", [128, 4 * 2048], F16, kind="ExternalOutput")

    with tile.TileContext(nc) as tc:
        with (
            tc.tile_pool(name="const", bufs=1) as constp,
            tc.tile_pool(name="xin", bufs=1) as xinp,
            tc.tile_pool(name="k1", bufs=2) as k1p,
            tc.tile_pool(name="sm", bufs=3) as smp,
            tc.tile_pool(name="ab", bufs=3) as abp,
            tc.tile_pool(name="ost", bufs=2) as ostp,
            tc.tile_pool(name="psA", bufs=2, space="PSUM") as psA,
            tc.tile_pool(name="psB", bufs=2, space="PSUM") as psB,
            tc.tile_pool(name="psX", bufs=2, space="PSUM") as psX,
        ):
            # ---- constants (issued between the first xq loads; see below) ----
            cpk = constp.tile([128, 496], F16)
            bpk = constp.tile([128, 101], F32)
            ixsb = constp.tile([128, NIDX], I16)
            w1sb = cpk[:, 0:128].rearrange("p (c m) -> p c m", c=2)
            idsb = cpk[:, 128:256]
            w2sb = cpk[0:M, 256:481].rearrange("p (t k) -> p t k", t=9)
            b2sb = bpk[:, 0:100].rearrange("p (h k) -> p h k", h=4)
            b1sb = bpk[0:M, 100:101]

            def load_consts_early():
                nc.sync.dma_start(out=cpk[:], in_=cpk_d.ap())
                nc.sync.dma_start(out=bpk[:], in_=bpk_d.ap())

            def load_consts_rest():
                nc.sync.dma_start(out=ixsb[:], in_=ix_d.ap())

            for _rep in range(repeat):
                # ---- per-rep state (software pipeline, stage offset 2) ----
                xt0 = [None] * 4
                xt1 = [None] * 4
                xq0 = [None] * 4
                xq1 = [None] * 4
                k1q = [None] * 4
                osbq = [None] * 4
                amats = {}            # sub sb -> amat tile

                def alloc_quarter(q):
                    xq0[q] = xinp.tile(
                        [128, 4096], F16, tag=f"xq0_{q % 2}", name=f"xq0_{q}"
                    )
                    xq1[q] = xinp.tile(
                        [128, 4096], F16, tag=f"xq1_{q % 2}", name=f"xq1_{q}"
                    )
                    xt0[q] = xinp.tile(
                        [128, 32, 128], F16, tag=f"xt0_{q}", name=f"xt0_{q}"
                    )
                    xt1[q] = xinp.tile(
                        [128, 32, 128], F16, tag=f"xt1_{q}", name=f"xt1_{q}"
                    )

                def load_half(q, h):
                    sl = slice(q * 4096 + h * 2048, q * 4096 + (h + 1) * 2048)
                    dl = slice(h * 2048, (h + 1) * 2048)
                    nc.sync.dma_start(out=xq0[q][:, dl], in_=xcm_d.ap()[0:128, sl])
                    nc.sync.dma_start(out=xq1[q][:, dl], in_=xcm_d.ap()[128:256, sl])

                def load_quarter(q):
                    alloc_quarter(q)
                    load_half(q, 0)
                    load_half(q, 1)

                xt_pend = []      # deferred PSUM->SBUF copies (pst, dst, alt)

                def flush_xt(keep=0):
                    while len(xt_pend) > keep:
                        pst, dst, _alt = xt_pend.pop(0)
                        nc.vector.tensor_copy(out=dst, in_=pst[:])

                def build_xt_T(r0, nr):
                    """Build xt for x rows [r0, r0+nr) (16-row units).

                    Most units go via PE transposes into fp16 PSUM batches
                    (PSUM->SBUF copies deferred ~1 batch); units with
                    (hf==1, cc==1) go via one xbar DMA transpose instead,
                    balancing PE vs the shared DMA pool."""
                    for cc in range(2):
                        xqc = xq0 if cc == 0 else xq1
                        xtc = xt0 if cc == 0 else xt1
                        for u0 in range(r0, r0 + nr, 16):
                            q, hf = u0 // 32, (u0 % 32) // 16
                            if hf == 1 and cc == 1:
                                nc.sync.dma_start(
                                    out=xtc[q][:, 16 * hf : 16 * hf + 16, :],
                                    in_=xqc[q][
                                        :, 2048 * hf : 2048 * (hf + 1)
                                    ],
                                    transpose=True,
                                )
                                continue
                            for b0 in range(u0, u0 + 16, 8):
                                flush_xt(keep=1)
                                pst = psX.tile(
                                    [128, 8, 128], F16, tag="xt", bufs=2
                                )
                                for k in range(8):
                                    r = b0 + k
                                    nc.tensor.transpose(
                                        pst[:, k, :],
                                        xqc[q][
                                            :,
                                            (r % 32) * 128 : (r % 32 + 1) * 128,
                                        ],
                                        idsb[:],
                                    )
                                dst = xtc[q][:, b0 % 32 : b0 % 32 + 8, :]
                                xt_pend.append((pst, dst, 0))

                def ensure_k1(q):
                    if k1q[q] is None:
                        k1q[q] = k1p.tile(
                            [M, 34, W + 2], F16, tag=f"k1_{q % 2}", name=f"k1t{q}"
                        )
                        nc.vector.memset(k1q[q][:, 0:33, 0], 0.0)
                        nc.vector.memset(k1q[q][:, 0:33, W + 1], 0.0)
                        if q == 0:
                            nc.vector.memset(k1q[0][:, 0, :], 0.0)
                    return k1q[q]

                def produce_sub(sb):
                    """compress half-quarter + encoder + softmax + scatter +
                    A-transpose for sub sb (4 output-row pairs)."""
                    q, half = sb // 2, sb % 2
                    k1c = ensure_k1(q)
                    for bl in range(4):     # 512-pixel (4-image-row) blocks
                        blk = 4 * half + bl
                        ps1 = psA.tile([M, 512], F32, tag="c", bufs=2)
                        nc.tensor.matmul(
                            ps1[:],
                            lhsT=w1sb[:, 0, :],
                            rhs=xq0[q][:, blk * 512 : (blk + 1) * 512],
                            start=True,
                            stop=False,
                        )
                        nc.tensor.matmul(
                            ps1[:],
                            lhsT=w1sb[:, 1, :],
                            rhs=xq1[q][:, blk * 512 : (blk + 1) * 512],
                            start=False,
                            stop=True,
                        )
                        # local padded rows l = blk*4+1 .. blk*4+4
                        dst = k1c[:, blk * 4 + 1 : blk * 4 + 5, 1 : 1 + W]
                        src = ps1[:].rearrange("p (r w) -> p r w", r=4)
                        nc.scalar.activation(
                            out=dst,
                            in_=src,
                            func=mybir.ActivationFunctionType.Identity,
                            bias=b1sb[:],
                        )
                        if blk == 7 and q < 3:
                            # halo: image row 32q+31 duplicated as next row 0
                            k1n = ensure_k1(q + 1)
                            nc.vector.tensor_scalar_add(
                                out=k1n[:, 0:1, 1 : 1 + W],
                                in0=ps1[:, 3 * 128 : 512].rearrange(
                                    "p (r w) -> p r w", r=1
                                ),
                                scalar1=b1sb[:],
                            )
                    # ---- encoder conv, transposed: psE[px, pair, kk] ----
                    k1v = k1c[:].rearrange(
                        "p (r s) (w t) -> p r s w t", s=2, t=2
                    )  # [64, 17, 2, 65, 2]
                    psE = psB.tile([128, 4, 28], F32, tag="e", bufs=2)
                    for pp in range(4):
                        l0base = 16 * half + 4 * pp
                        for s in range(2):
                            for tap in range(9):
                                dy, dx = tap // 3, tap % 3
                                l = l0base + 2 * s + dy
                                # single-free-dim stationary operand (HW rule)
                                lhsT = k1v[
                                    :,
                                    l // 2,
                                    l & 1,
                                    dx // 2 : dx // 2 + nW,
                                    dx & 1,
                                ]
                                nc.tensor.matmul(
                                    psE[64 * s : 64 * s + 64, pp, 0:KK],
                                    lhsT=lhsT,
                                    rhs=w2sb[:, tap, :],
                                    start=(tap == 0),
                                    stop=(tap == 8),
                                    # sim's zero-region tracker ignores the
                                    # partition offset; s=0/s=1 chains are on
                                    # disjoint partitions (HW-legal, verified)
                                    skip_group_check=True,
                                )
                    # ---- softmax (b2 added via const tile) ----
                    eb = smp.tile([128, 4, KK], F32, tag="eb")
                    nc.vector.tensor_tensor(
                        out=eb[:],
                        in0=psE[:, :, 0:KK],
                        in1=b2sb[:],
                        op=mybir.AluOpType.add,
                    )
                    ecb = smp.tile([128, 4, KK], F16, tag="e")
                    nc.scalar.activation(
                        out=ecb[:],
                        in_=eb[:],
                        func=mybir.ActivationFunctionType.Exp,
                    )
                    scb = smp.tile([128, 4], F32, tag="s")
                    nc.vector.reduce_sum(
                        out=scb[:], in_=ecb[:], axis=mybir.AxisListType.X
                    )
                    rcb = smp.tile([128, 4], F32, tag="r")
                    nc.vector.reciprocal(out=rcb[:], in_=scb[:])
                    wn = smp.tile([128, NIDX], F16, tag="wn", bufs=3)
                    wnv = wn[:].rearrange("p (h k) -> p h k", h=4)
                    nc.vector.memset(wnv[:, :, 25:26], 0.0)
                    for h in range(4):
                        nc.vector.tensor_scalar_mul(
                            out=wn[:, 26 * h : 26 * h + KK],
                            in0=ecb[:, h, :],
                            scalar1=rcb[:, h : h + 1],
                        )
                    ams = []
                    for gg in range(2):
                        at2 = smp.tile([128, 2 * GSLOT], F16, tag="at", bufs=3)
                        nc.gpsimd.local_scatter(
                            out_ap=at2[:],
                            data_ap=wn[:, 52 * gg : 52 * (gg + 1)],
                            idxs_ap=ixsb[:, 52 * gg : 52 * (gg + 1)],
                            channels=128,
                            num_elems=2 * GSLOT,
                            num_idxs=NIDX // 2,
                        )
                        amat = abp.tile(
                            [128, 2 * QCH, 128], F16, tag="A", bufs=6
                        )
                        nc.sync.dma_start(out=amat[:], in_=at2[:], transpose=True)
                        ams.append(amat)
                    amats[sb] = ams

                def consume_sub(sb):
                    """weighted sum + output copy for sub sb's two groups."""
                    q, half = sb // 2, sb % 2
                    if half == 0:
                        osbq[q] = ostp.tile(
                            [128, 2048], F16, tag="osb", name=f"osb{q}"
                        )
                    osb = osbq[q]
                    ams = amats.pop(sb)
                    for gg in range(2):
                        amat = ams[gg]
                        g = 2 * sb + gg
                        P0 = 2 * g
                        psF = psA.tile([128, 512], F32, tag="f", bufs=2)
                        for pr in range(2):
                            P = P0 + pr
                            # band rows 0,1 only touch out cols 0:64 (s=0);
                            # rows 5,6 only 64:128 (s=1); rows 2,3,4 full.
                            # qq=2 (row 4P, always in-image) leads as start.
                            specs = [(2, 0, 128)]
                            for qq in (0, 1):
                                if 4 * P - 2 + qq >= 0:
                                    specs.append((qq, 0, 64))
                            specs += [(3, 0, 128), (4, 0, 128)]
                            for qq in (5, 6):
                                if 4 * P - 2 + qq < H:
                                    specs.append((qq, 64, 128))
                            for cc in range(2):
                                base = (2 * pr + cc) * 128
                                xtc = xt0 if cc == 0 else xt1
                                for qi, (qq, c0, c1) in enumerate(specs):
                                    r = 4 * P - 2 + qq
                                    tl = r // 32
                                    nc.tensor.matmul(
                                        psF[:, base + c0 : base + c1],
                                        lhsT=xtc[tl][:, r - 32 * tl, :],
                                        rhs=amat[:, QCH * pr + qq, c0:c1],
                                        start=(qi == 0),
                                        stop=(qi == len(specs) - 1),
                                    )
                        gl = g % 4
                        nc.scalar.copy(
                            out=osb[:, gl * 512 : (gl + 1) * 512], in_=psF[:]
                        )

                alloc_quarter(0)
                load_half(0, 0)
                load_consts_early()
                load_half(0, 1)
                load_consts_rest()
                load_quarter(1)
                build_xt_T(0, 32)
                for sb in range(10):
                    if sb < 8:
                        produce_sub(sb)
                    if sb < 6:
                        build_xt_T(32 + 16 * sb, 16)
                    flush_xt(keep=1 if sb < 6 else 0)
                    if sb >= 2:
                        consume_sub(sb - 2)
                        # store this sub's 1024-col osb slice right away
                        s = sb - 2
                        q, half = s // 2, s % 2
                        nc.sync.dma_start(
                            out=o_d.ap()[
                                :, q * 2048 + half * 1024 : q * 2048 + (half + 1) * 1024
                            ],
                            in_=osbq[q][:, half * 1024 : (half + 1) * 1024],
                        )
                    if sb % 2 == 1 and sb < 4:
                        load_quarter(sb // 2 + 2)
                flush_xt()

    nc.compile()
    return nc


_NC = None


def _get_nc():
    global _NC
    if _NC is None:
        _NC = _build_program()
    return _NC


def _host_consts(w1, b1, w2, b2):
    """Precompute packed constant tensors (host-side, numpy)."""
    w1m = np.asarray(w1, np.float32).reshape(M, C)            # [m, c]
    w1t = np.transpose(w1m.reshape(M, 2, 128), (2, 1, 0)).astype(np.float16)
    w2m = np.asarray(w2, np.float32).reshape(KK, M, 9)        # [k, m, tap]
    w2t = np.transpose(w2m, (1, 2, 0)).astype(np.float16)    # [m, tap, k]
    cpk = np.zeros((128, 496), np.float16)
    cpk[:, 0:128] = w1t.reshape(128, 128)
    cpk[:, 128:256] = np.eye(128, dtype=np.float16)
    cpk[0:M, 256:481] = w2t.reshape(M, 9 * KK)
    bpk = np.zeros((128, 101), np.float32)
    bpk[:, 0:100] = np.broadcast_to(
        np.asarray(b2, np.float32)[None, None, :], (128, 4, KK)
    ).reshape(128, 100)
    bpk[0:M, 100] = np.asarray(b1, np.float32)
    scidx = _exp_taps5()
    return {"cpk": cpk, "bpk": bpk, "scidx": scidx}


def make_in_maps(x, w1, b1, w2, b2):
    """Per-core input dicts: host-side layout prep (fp16)."""
    x = np.asarray(x, np.float32)
    consts = _host_consts(w1, b1, w2, b2)
    xf = x.astype(np.float16)                                  # [B, C, H, W]
    in_maps = []
    for b in range(B):
        m = {"xcm": np.ascontiguousarray(xf[b].reshape(C, NPIX))}
        m.update(consts)
        in_maps.append(m)
    return in_maps


def unscramble(res_list):
    """[B] x out[128, 8192] fp16 -> [B, C, nH, nW] fp32.

    out dram layout: [p, q, gl, pr, cc, s, ow]; c = cc*128 + p,
    oh = 16q + 4gl + 2pr + s.
    """
    out = np.stack(res_list, axis=0).reshape(B, 128, 4, 4, 2, 2, 2, 64)
    out = np.transpose(out, (0, 5, 1, 2, 3, 4, 6, 7))  # b,cc,p,q,gl,pr,s,ow
    return np.ascontiguousarray(out).reshape(B, C, nH, nW).astype(np.float32)


def kernel(x, w1, b1, w2, b2):
    in_maps = make_in_maps(x, w1, b1, w2, b2)
    nc = _get_nc()
    res = run_bass_kernel_spmd(nc, in_maps, core_ids=list(range(B)))
    return unscramble([res.results[i]["out"] for i in range(B)])
